# revision 13
# baseline (speedup 1.0000x reference)
"""DiffLogicLayer Trainium2 kernel.

Math: for each output neuron o with inputs a = x[:, ia[o]], b = x[:, ib[o]],
the 16 relaxed binary gates are all linear in {1, a, b, a*b}:

    gate_k(a, b) = C[k,0] + C[k,1]*a + C[k,2]*b + C[k,3]*a*b

so with w = softmax(weights[o]) the layer output collapses to

    out[n, o] = W0[o] + W1[o]*a + W2[o]*b + W3[o]*a*b,   W = softmax(weights) @ C

Sharding: tensor-parallel over out_dim (1024 neurons/core). Each neuron reads
exactly two x columns, so the shard handed to core c is those columns,
pre-gathered and interleaved per 128-neuron block (a-rows then b-rows), in
fp16. That keeps the device streams dense: 8 MB in + 4 MB out per core, all
on HWDGE — no on-device gather, no GPSIMD ucode preamble. fp16 quantization
of a/b/out gives max rel err ~4e-3 vs the f32 reference (gate is 2e-2).

Device kernel (per core):
  - softmax + C-fold of this core's (1024, 16) weight slice (ACT+DVE, runs
    under the first input DMA)
  - per 128-neuron block j: stream (128, 2*2048) fp16 a|b tile, then
    u = W3*a + W2 (ACT), v = W1*a + W0 (DVE tensor_scalar, 4x mode),
    t = u*b (DVE), o = t + v (DVE, fp16 2x mode); DMA o to DRAM in fp16.

Host only reshapes/gathers (sharding prep), concatenates shards, and
upcasts the fp16 output to f32.
"""

import os
import sys

import numpy as np

sys.path.insert(0, "/opt/trn_rl_repo")

import concourse.bacc as bacc
import concourse.mybir as mybir
from concourse import tile
from concourse.bass_utils import run_bass_kernel_spmd

AF = mybir.ActivationFunctionType
ALU = mybir.AluOpType
AX = mybir.AxisListType
F32 = mybir.dt.float32
F16 = mybir.dt.float16

IN_DIM = 8192
OUT_DIM = 8192
BATCH = 2048
N_CORES = 8
OPC = OUT_DIM // N_CORES  # 1024 neurons per core
NBLK = OPC // 128  # 8 partition blocks per core

# gate_k = C[k,0] + C[k,1]*a + C[k,2]*b + C[k,3]*ab  (difflogic convention)
_C = np.array(
    [
        [0, 0, 0, 0],  # False
        [0, 0, 0, 1],  # a AND b
        [0, 1, 0, -1],  # a AND NOT b
        [0, 1, 0, 0],  # a
        [0, 0, 1, -1],  # NOT a AND b
        [0, 0, 1, 0],  # b
        [0, 1, 1, -2],  # XOR
        [0, 1, 1, -1],  # OR
        [1, -1, -1, 1],  # NOR
        [1, -1, -1, 2],  # XNOR
        [1, 0, -1, 0],  # NOT b
        [1, 0, -1, 1],  # a OR NOT b
        [1, -1, 0, 0],  # NOT a
        [1, -1, 0, 1],  # NOT a OR b
        [1, 0, 0, -1],  # NAND
        [1, 0, 0, 0],  # True
    ],
    dtype=np.float32,
)

_PROGRAM = None


def _build_program():
    nc = bacc.Bacc("TRN2", target_bir_lowering=False, debug=False)

    ab = nc.dram_tensor("ab", (128, NBLK * 2 * BATCH), F16, kind="ExternalInput")
    wpre = nc.dram_tensor("wpre", (128, NBLK * 16), F32, kind="ExternalInput")
    yt = nc.dram_tensor("yt", (OPC, BATCH), F16, kind="ExternalOutput")

    with tile.TileContext(nc) as tc:
        with (
            tc.tile_pool(name="const", bufs=1) as cpool,
            tc.tile_pool(name="gath", bufs=NBLK // 2) as gpool,
            tc.tile_pool(name="work", bufs=2) as wpool,
        ):
            # Weight loads go FIRST on the SP ring: they are tiny (0.3 MB) and
            # gate the softmax -> w4 chain, which gates all block compute. On
            # a separate ring they'd round-robin with the big input stream and
            # complete ~8us late (measured), stalling the whole pipeline.
            wpre_t = cpool.tile([128, NBLK * 16], F32)
            nc.sync.dma_start(wpre_t[:, :], wpre[:, :])

            # softmax over the 16 gate logits of each neuron, then fold with C:
            # w4[:, c*NBLK + j] = sum_k softmax(w)[p + 128j, k] * C[k, c].
            # Writing k = 8h + 4q + 2r + s, the difflogic C columns factor as
            #   C0 = h,  C1 = r - h,  C2 = q - h,  C3 = (s - r) + (h - q)
            # so the fold needs only 4 strided partial sums of exp(w) — no
            # constant tensor load.
            e_t = cpool.tile([128, NBLK * 16], F32)
            nc.scalar.activation(e_t[:, :], wpre_t[:, :], AF.Exp)
            s_t = cpool.tile([128, NBLK], F32)
            nc.vector.tensor_reduce(
                s_t[:, :], e_t[:, :].rearrange("p (j k) -> p j k", k=16), AX.X, op=ALU.add
            )
            r_t = cpool.tile([128, NBLK], F32)
            nc.vector.reciprocal(r_t[:, :], s_t[:, :])

            def psum(tag, grp, lo, hi):
                # sum_k e[p, j*16+k] over k with (k mod grp) in [lo, hi):
                # view (p, j, 16/grp, grp), slice the last axis, reduce XY.
                out = cpool.tile([128, NBLK], F32, tag=tag)
                v = e_t[:, :].rearrange("p (j m d) -> p j m d", m=16 // grp, d=grp)
                nc.vector.tensor_reduce(out[:, :], v[:, :, :, lo:hi], AX.XY, op=ALU.add)
                return out

            sh_t = psum("sh", 16, 8, 16)  # h=1: k in 8..15
            sq_t = psum("sq", 8, 4, 8)  # q=1: k mod 8 in 4..7
            sr_t = psum("sr", 4, 2, 4)  # r=1: k mod 4 in 2..3
            ss_t = psum("ss", 2, 1, 2)  # s=1: k odd

            w4_t = cpool.tile([128, 4 * NBLK], F32)
            d_t = cpool.tile([128, NBLK], F32, tag="d")
            d2_t = cpool.tile([128, NBLK], F32, tag="d2")
            # u's coefficients (c=2 bias, c=3 scale) first so block compute
            # can start as early as possible.
            nc.vector.tensor_tensor(d_t[:, :], sq_t[:, :], sh_t[:, :], op=ALU.subtract)
            nc.vector.tensor_tensor(
                w4_t[:, 2 * NBLK : 3 * NBLK], d_t[:, :], r_t[:, :], op=ALU.mult
            )
            d3_t = cpool.tile([128, NBLK], F32, tag="d3")
            nc.vector.tensor_tensor(d2_t[:, :], ss_t[:, :], sr_t[:, :], op=ALU.subtract)
            nc.vector.tensor_tensor(d3_t[:, :], d2_t[:, :], d_t[:, :], op=ALU.subtract)
            nc.vector.tensor_tensor(
                w4_t[:, 3 * NBLK : 4 * NBLK], d3_t[:, :], r_t[:, :], op=ALU.mult
            )
            d4_t = cpool.tile([128, NBLK], F32, tag="d4")
            nc.vector.tensor_tensor(d4_t[:, :], sr_t[:, :], sh_t[:, :], op=ALU.subtract)
            nc.vector.tensor_tensor(
                w4_t[:, 1 * NBLK : 2 * NBLK], d4_t[:, :], r_t[:, :], op=ALU.mult
            )
            nc.vector.tensor_tensor(
                w4_t[:, 0:NBLK], sh_t[:, :], r_t[:, :], op=ALU.mult
            )

            def wc(c, j):
                return w4_t[:, c * NBLK + j : c * NBLK + j + 1]

            # All 8 input DMAs are issued up-front so the SP HWDGE ring (FIFO)
            # drains the whole 8 MB input stream back-to-back; output DMAs
            # queue behind it and drain at full rate at the end. Interleaving
            # outs between ins (or a second ring) makes the last input land
            # several us later (measured).
            gs = []
            for j2 in range(NBLK // 2):
                g_t = gpool.tile([128, 4 * BATCH], F16, tag="g")
                nc.sync.dma_start(g_t[:, :], ab[:, j2 * 4 * BATCH : (j2 + 1) * 4 * BATCH])
                gs.append(g_t)
            for j in range(NBLK):
                g_t = gs[j // 2]
                off = (j % 2) * 2 * BATCH
                a_ap = g_t[:, off : off + BATCH]
                b_ap = g_t[:, off + BATCH : off + 2 * BATCH]
                u_t = wpool.tile([128, BATCH], F16, tag="u")
                nc.scalar.activation(
                    u_t[:, :], a_ap, AF.Identity, bias=wc(2, j), scale=wc(3, j)
                )
                v_t = wpool.tile([128, BATCH], F16, tag="v")
                if j >= NBLK - 2:
                    # ACT has ~10us of slack; taking the last two v's off DVE
                    # lets the DVE chain (which gates the final output DMA)
                    # finish earlier.
                    nc.scalar.activation(
                        v_t[:, :], a_ap, AF.Identity, bias=wc(0, j), scale=wc(1, j)
                    )
                else:
                    nc.vector.tensor_scalar(
                        v_t[:, :], a_ap, wc(1, j), wc(0, j), op0=ALU.mult, op1=ALU.add
                    )
                t_t = wpool.tile([128, BATCH], F16, tag="t")
                nc.vector.tensor_tensor(t_t[:, :], u_t[:, :], b_ap, op=ALU.mult)
                # o tiles are not recycled (bufs=NBLK): recycling would make
                # block j+2 compute wait on output-DMA completion, which only
                # happens after the entire input stream has drained.
                o_t = wpool.tile([128, BATCH], F16, tag="o", bufs=NBLK)
                nc.vector.tensor_tensor(o_t[:, :], t_t[:, :], v_t[:, :], op=ALU.add)
                nc.sync.dma_start(yt[j * 128 : (j + 1) * 128, :], o_t[:, :])

    nc.compile()
    return nc


def _get_program():
    global _PROGRAM
    if _PROGRAM is None:
        _PROGRAM = _build_program()
    return _PROGRAM


def make_in_maps(x, weights, indices_a, indices_b):
    x = np.asarray(x, dtype=np.float32)
    w = np.asarray(weights, dtype=np.float32)
    ia = np.asarray(indices_a).astype(np.int64)
    ib = np.asarray(indices_b).astype(np.int64)

    xt16 = np.ascontiguousarray(x.T).astype(np.float16)  # (IN_DIM, BATCH)

    in_maps = []
    for c in range(N_CORES):
        sl = slice(c * OPC, (c + 1) * OPC)
        # big[p, j, 0] = ia of neuron j*128+p on this core; big[p, j, 1] = ib
        ia_c = ia[sl].reshape(NBLK, 128)
        ib_c = ib[sl].reshape(NBLK, 128)
        big = np.stack([ia_c.T, ib_c.T], axis=2)  # (128, NBLK, 2)
        ab_c = np.ascontiguousarray(xt16[big].reshape(128, NBLK * 2 * BATCH))
        wsh = w[sl]  # (OPC, 16)
        wpre = np.ascontiguousarray(
            wsh.reshape(NBLK, 128, 16).transpose(1, 0, 2).reshape(128, NBLK * 16)
        )
        in_maps.append({"ab": ab_c, "wpre": wpre})
    return in_maps


def run(inputs, trace=False):
    if trace:
        try:
            from antenv.axon_hooks import get_axon_ntff_profile_hook  # noqa: F401
        except ImportError:
            trace = False
    nc = _get_program()
    in_maps = make_in_maps(
        inputs["x"], inputs["weights"], inputs["indices_a"], inputs["indices_b"]
    )
    res = run_bass_kernel_spmd(nc, in_maps, core_ids=list(range(N_CORES)), trace=trace)
    outT = np.empty((OUT_DIM, BATCH), dtype=np.float16)
    for c in range(N_CORES):
        outT[c * OPC : (c + 1) * OPC] = res.results[c]["yt"]
    return outT.T.astype(np.float32), res


def kernel(**inputs):
    out, _ = run(inputs, trace=bool(os.environ.get("DL_TRACE")))
    return out


if __name__ == "__main__":
    rng = np.random.default_rng(0)
    inputs = {
        "x": rng.random((BATCH, IN_DIM), dtype=np.float32),
        "weights": rng.standard_normal((OUT_DIM, 16)).astype(np.float32),
        "indices_a": rng.integers(0, IN_DIM, size=OUT_DIM),
        "indices_b": rng.integers(0, IN_DIM, size=OUT_DIM),
    }
    out = kernel(**inputs)
    print(out.shape, out.dtype)


# revision 15
# speedup vs baseline: 1.1273x; 1.1273x over previous
"""DiffLogicLayer Trainium2 kernel.

Math: for each output neuron o with inputs a = x[:, ia[o]], b = x[:, ib[o]],
the 16 relaxed binary gates are all linear in {1, a, b, a*b}:

    gate_k(a, b) = C[k,0] + C[k,1]*a + C[k,2]*b + C[k,3]*a*b

so with w = softmax(weights[o]) the layer output collapses to

    out[n, o] = W0[o] + W1[o]*a + W2[o]*b + W3[o]*a*b,   W = softmax(weights) @ C

Sharding: tensor-parallel over out_dim (1024 neurons/core). Each neuron reads
exactly two x columns, so the shard handed to core c is those columns,
pre-gathered and interleaved per 128-neuron block (a-rows then b-rows), in
fp16. That keeps the device streams dense: 8 MB in + 4 MB out per core, all
on HWDGE — no on-device gather, no GPSIMD ucode preamble. fp16 quantization
of a/b/out gives max rel err ~4e-3 vs the f32 reference (gate is 2e-2).

Device kernel (per core):
  - softmax + C-fold of this core's (1024, 16) weight slice (ACT+DVE, runs
    under the first input DMA)
  - per 128-neuron block j: stream (128, 2*2048) fp16 a|b tile, then
    u = W3*a + W2 (ACT), v = W1*a + W0 (DVE tensor_scalar, 4x mode),
    t = u*b (DVE), o = t + v (DVE, fp16 2x mode); DMA o to DRAM in fp16.

Host only reshapes/gathers (sharding prep), concatenates shards, and
upcasts the fp16 output to f32.
"""

import os
import sys

import numpy as np

sys.path.insert(0, "/opt/trn_rl_repo")

import concourse.bacc as bacc
import concourse.mybir as mybir
from concourse import tile
from concourse.bass_utils import run_bass_kernel_spmd

AF = mybir.ActivationFunctionType
ALU = mybir.AluOpType
AX = mybir.AxisListType
F32 = mybir.dt.float32
F16 = mybir.dt.float16

IN_DIM = 8192
OUT_DIM = 8192
BATCH = 2048
N_CORES = 8
OPC = OUT_DIM // N_CORES  # 1024 neurons per core
NBLK = OPC // 128  # 8 partition blocks per core

# gate_k = C[k,0] + C[k,1]*a + C[k,2]*b + C[k,3]*ab  (difflogic convention)
_C = np.array(
    [
        [0, 0, 0, 0],  # False
        [0, 0, 0, 1],  # a AND b
        [0, 1, 0, -1],  # a AND NOT b
        [0, 1, 0, 0],  # a
        [0, 0, 1, -1],  # NOT a AND b
        [0, 0, 1, 0],  # b
        [0, 1, 1, -2],  # XOR
        [0, 1, 1, -1],  # OR
        [1, -1, -1, 1],  # NOR
        [1, -1, -1, 2],  # XNOR
        [1, 0, -1, 0],  # NOT b
        [1, 0, -1, 1],  # a OR NOT b
        [1, -1, 0, 0],  # NOT a
        [1, -1, 0, 1],  # NOT a OR b
        [1, 0, 0, -1],  # NAND
        [1, 0, 0, 0],  # True
    ],
    dtype=np.float32,
)

_PROGRAM = None


def _build_program():
    nc = bacc.Bacc("TRN2", target_bir_lowering=False, debug=False)

    ab = nc.dram_tensor("ab", (128, NBLK * 2 * BATCH), F16, kind="ExternalInput")
    wpre = nc.dram_tensor("wpre", (128, NBLK * 16), F32, kind="ExternalInput")
    yt = nc.dram_tensor("yt", (OPC, BATCH), F16, kind="ExternalOutput")

    with tile.TileContext(nc) as tc:
        with (
            tc.tile_pool(name="const", bufs=1) as cpool,
            tc.tile_pool(name="gath", bufs=NBLK) as gpool,
            tc.tile_pool(name="work", bufs=2) as wpool,
        ):
            # Weight loads go FIRST on the SP ring: they are tiny (0.3 MB) and
            # gate the softmax -> w4 chain, which gates all block compute. On
            # a separate ring they'd round-robin with the big input stream and
            # complete ~8us late (measured), stalling the whole pipeline.
            wpre_t = cpool.tile([128, NBLK * 16], F32)
            nc.sync.dma_start(wpre_t[:, :], wpre[:, :])

            # softmax over the 16 gate logits of each neuron, then fold with C:
            # w4[:, c*NBLK + j] = sum_k softmax(w)[p + 128j, k] * C[k, c].
            # Writing k = 8h + 4q + 2r + s, the difflogic C columns factor as
            #   C0 = h,  C1 = r - h,  C2 = q - h,  C3 = (s - r) + (h - q)
            # so the fold needs only 4 strided partial sums of exp(w) — no
            # constant tensor load.
            e_t = cpool.tile([128, NBLK * 16], F32)
            nc.scalar.activation(e_t[:, :], wpre_t[:, :], AF.Exp)
            s_t = cpool.tile([128, NBLK], F32)
            nc.vector.tensor_reduce(
                s_t[:, :], e_t[:, :].rearrange("p (j k) -> p j k", k=16), AX.X, op=ALU.add
            )
            r_t = cpool.tile([128, NBLK], F32)
            nc.vector.reciprocal(r_t[:, :], s_t[:, :])

            def psum(tag, grp, lo, hi):
                # sum_k e[p, j*16+k] over k with (k mod grp) in [lo, hi):
                # view (p, j, 16/grp, grp), slice the last axis, reduce XY.
                out = cpool.tile([128, NBLK], F32, tag=tag)
                v = e_t[:, :].rearrange("p (j m d) -> p j m d", m=16 // grp, d=grp)
                nc.vector.tensor_reduce(out[:, :], v[:, :, :, lo:hi], AX.XY, op=ALU.add)
                return out

            sh_t = psum("sh", 16, 8, 16)  # h=1: k in 8..15
            sq_t = psum("sq", 8, 4, 8)  # q=1: k mod 8 in 4..7
            sr_t = psum("sr", 4, 2, 4)  # r=1: k mod 4 in 2..3
            ss_t = psum("ss", 2, 1, 2)  # s=1: k odd

            w4_t = cpool.tile([128, 4 * NBLK], F32)
            d_t = cpool.tile([128, NBLK], F32, tag="d")
            d2_t = cpool.tile([128, NBLK], F32, tag="d2")
            # u's coefficients (c=2 bias, c=3 scale) first so block compute
            # can start as early as possible.
            nc.vector.tensor_tensor(d_t[:, :], sq_t[:, :], sh_t[:, :], op=ALU.subtract)
            nc.vector.tensor_tensor(
                w4_t[:, 2 * NBLK : 3 * NBLK], d_t[:, :], r_t[:, :], op=ALU.mult
            )
            d3_t = cpool.tile([128, NBLK], F32, tag="d3")
            nc.vector.tensor_tensor(d2_t[:, :], ss_t[:, :], sr_t[:, :], op=ALU.subtract)
            nc.vector.tensor_tensor(d3_t[:, :], d2_t[:, :], d_t[:, :], op=ALU.subtract)
            nc.vector.tensor_tensor(
                w4_t[:, 3 * NBLK : 4 * NBLK], d3_t[:, :], r_t[:, :], op=ALU.mult
            )
            d4_t = cpool.tile([128, NBLK], F32, tag="d4")
            nc.vector.tensor_tensor(d4_t[:, :], sr_t[:, :], sh_t[:, :], op=ALU.subtract)
            nc.vector.tensor_tensor(
                w4_t[:, 1 * NBLK : 2 * NBLK], d4_t[:, :], r_t[:, :], op=ALU.mult
            )
            nc.vector.tensor_tensor(
                w4_t[:, 0:NBLK], sh_t[:, :], r_t[:, :], op=ALU.mult
            )

            def wc(c, j):
                return w4_t[:, c * NBLK + j : c * NBLK + j + 1]

            # All 8 input DMAs are issued up-front so the SP HWDGE ring (FIFO)
            # drains the whole 8 MB input stream back-to-back; output DMAs
            # queue behind it and drain at full rate at the end. Interleaving
            # outs between ins (or a second ring) makes the last input land
            # several us later (measured).
            # 1 MB per input DMA: coarser chunks (2 MB) delay the first block's
            # data ~3us and shift the whole compute stream right (measured).
            gs = []
            for j in range(NBLK):
                g_t = gpool.tile([128, 2 * BATCH], F16, tag="g")
                nc.sync.dma_start(g_t[:, :], ab[:, j * 2 * BATCH : (j + 1) * 2 * BATCH])
                gs.append(g_t)
            for j in range(NBLK):
                g_t = gs[j]
                a_ap = g_t[:, 0:BATCH]
                b_ap = g_t[:, BATCH : 2 * BATCH]
                u_t = wpool.tile([128, BATCH], F16, tag="u")
                nc.scalar.activation(
                    u_t[:, :], a_ap, AF.Identity, bias=wc(2, j), scale=wc(3, j)
                )
                # v stays on DVE: putting it on ACT serializes with u on the
                # same engine and adds ~2us per affected block at the tail
                # (measured regression).
                v_t = wpool.tile([128, BATCH], F16, tag="v")
                nc.vector.tensor_scalar(
                    v_t[:, :], a_ap, wc(1, j), wc(0, j), op0=ALU.mult, op1=ALU.add
                )
                t_t = wpool.tile([128, BATCH], F16, tag="t")
                nc.vector.tensor_tensor(t_t[:, :], u_t[:, :], b_ap, op=ALU.mult)
                # o tiles are not recycled (bufs=NBLK): recycling would make
                # block j+2 compute wait on output-DMA completion, which only
                # happens after the entire input stream has drained.
                o_t = wpool.tile([128, BATCH], F16, tag="o", bufs=NBLK)
                nc.vector.tensor_tensor(o_t[:, :], t_t[:, :], v_t[:, :], op=ALU.add)
                nc.sync.dma_start(yt[j * 128 : (j + 1) * 128, :], o_t[:, :])

    nc.compile()
    return nc


def _get_program():
    global _PROGRAM
    if _PROGRAM is None:
        _PROGRAM = _build_program()
    return _PROGRAM


def make_in_maps(x, weights, indices_a, indices_b):
    x = np.asarray(x, dtype=np.float32)
    w = np.asarray(weights, dtype=np.float32)
    ia = np.asarray(indices_a).astype(np.int64)
    ib = np.asarray(indices_b).astype(np.int64)

    xt16 = np.ascontiguousarray(x.T).astype(np.float16)  # (IN_DIM, BATCH)

    in_maps = []
    for c in range(N_CORES):
        sl = slice(c * OPC, (c + 1) * OPC)
        # big[p, j, 0] = ia of neuron j*128+p on this core; big[p, j, 1] = ib
        ia_c = ia[sl].reshape(NBLK, 128)
        ib_c = ib[sl].reshape(NBLK, 128)
        big = np.stack([ia_c.T, ib_c.T], axis=2)  # (128, NBLK, 2)
        ab_c = np.ascontiguousarray(xt16[big].reshape(128, NBLK * 2 * BATCH))
        wsh = w[sl]  # (OPC, 16)
        wpre = np.ascontiguousarray(
            wsh.reshape(NBLK, 128, 16).transpose(1, 0, 2).reshape(128, NBLK * 16)
        )
        in_maps.append({"ab": ab_c, "wpre": wpre})
    return in_maps


def run(inputs, trace=False):
    if trace:
        try:
            from antenv.axon_hooks import get_axon_ntff_profile_hook  # noqa: F401
        except ImportError:
            trace = False
    nc = _get_program()
    in_maps = make_in_maps(
        inputs["x"], inputs["weights"], inputs["indices_a"], inputs["indices_b"]
    )
    res = run_bass_kernel_spmd(nc, in_maps, core_ids=list(range(N_CORES)), trace=trace)
    outT = np.empty((OUT_DIM, BATCH), dtype=np.float16)
    for c in range(N_CORES):
        outT[c * OPC : (c + 1) * OPC] = res.results[c]["yt"]
    return outT.T.astype(np.float32), res


def kernel(**inputs):
    out, _ = run(inputs, trace=bool(os.environ.get("DL_TRACE")))
    return out


if __name__ == "__main__":
    rng = np.random.default_rng(0)
    inputs = {
        "x": rng.random((BATCH, IN_DIM), dtype=np.float32),
        "weights": rng.standard_normal((OUT_DIM, 16)).astype(np.float32),
        "indices_a": rng.integers(0, IN_DIM, size=OUT_DIM),
        "indices_b": rng.integers(0, IN_DIM, size=OUT_DIM),
    }
    out = kernel(**inputs)
    print(out.shape, out.dtype)
